# revision 1
# baseline (speedup 1.0000x reference)
"""MAGNN aggregation kernel for 8 Trainium2 NeuronCores.

Split design: the host performs the irregular edge gather / segment-mean
stages as CSR SpMM (scipy sparsetools, zero-alloc into preallocated
buffers); the 8 NeuronCores run an SPMD Bass/Tile kernel computing the
dense epilogue for their node shard:
    y_k = relu(s_k @ W_k.T + b_k)      k in {1,2,12}
    sc_k = <y_k, att_k>,  w = softmax(sc),  out = sum_k w_k * y_k

Wall-clock critical choices:
  - All large host buffers preallocated + page-warmed once (page faults
    on this box are ~135 MB/s; warm streaming is GB/s).
  - scatter_mean == diag(1/cnt) @ CSR @ X  (csr_matvecs, ~0.2 s/SpMM vs
    ~15 s for the argsort+reduceat formulation).
  - Device I/O in bf16 (halves the ~50 MB/s axon tunnel traffic); node
    shards ship node-major with zero host packing and are transposed by
    the DMA xbar on load.
  - The PJRT dispatch (jit of the bass custom call) is built ONCE and
    cached; inputs are device_put asynchronously as each host SpMM
    completes so transfer overlaps host compute; the donated output
    buffers are generated on-device (never shipped); device buffers are
    freed eagerly so dealloc chatter can't stall the next call; a
    persistent XLA compilation cache makes fresh-process cold starts
    cheap.
"""
import os
import numpy as np
import ml_dtypes

BF16 = ml_dtypes.bfloat16

P = 128
D = 128
NCORES = 8
N0, N1, N2 = 100000, 50000, 50000
N0P = 100352                 # 8 * 12544
ROWS = N0P // NCORES         # 12544 rows per core
GB = 512                     # node columns per group

# 12544 = 24*512 + 256 : last group is half-width
GROUPS = [(g * GB, GB) for g in range(ROWS // GB)]
if ROWS % GB:
    GROUPS.append((ROWS - ROWS % GB, ROWS % GB))

# Independent sub-mesh dispatches per call. 2 is the sweet spot here:
# it lets split 0's output fetch overlap split 1's input puts (the axon
# tunnel is full-duplex), while keeping the per-dispatch CPU overhead
# (~0.1 s each on this single-core host) amortized. NSPLIT=4 measured
# slower (dispatch+put overhead outgrew the extra overlap).
NSPLIT = 2

_C = {}                      # program / dispatch / host-state cache
LAST_EXEC_NS = None


# --------------------------------------------------------------------------
# device program
# --------------------------------------------------------------------------

def _build_program():
    import concourse.bacc as bacc
    import concourse.mybir as mybir
    import concourse.tile as tile

    nc = bacc.Bacc("TRN2", target_bir_lowering=False, debug=False,
                   num_devices=NCORES)
    bf = mybir.dt.bfloat16
    f32 = mybir.dt.float32
    sD = [nc.dram_tensor(f"s{k}", [ROWS, D], bf, kind="ExternalInput")
          for k in range(3)]
    wt = nc.dram_tensor("wt", [P, 3 * D], bf, kind="ExternalInput")
    bias = nc.dram_tensor("bias", [P, 3], f32, kind="ExternalInput")
    att = nc.dram_tensor("att", [P, 3], bf, kind="ExternalInput")
    outT = nc.dram_tensor("outT", [P, ROWS], bf, kind="ExternalOutput")
    Relu = mybir.ActivationFunctionType.Relu
    Exp = mybir.ActivationFunctionType.Exp
    Mult = mybir.AluOpType.mult
    Add = mybir.AluOpType.add

    with tile.TileContext(nc) as tc:
        with tc.tile_pool(name="sb", bufs=2) as sb, \
             tc.tile_pool(name="cst", bufs=1) as cst, \
             tc.tile_pool(name="ps", bufs=1, space="PSUM") as ps:
            wt_t = cst.tile([P, 3 * D], bf)
            nc.sync.dma_start(out=wt_t[:], in_=wt[:])
            b_t = cst.tile([P, 3], f32)
            nc.sync.dma_start(out=b_t[:], in_=bias[:])
            a_t = cst.tile([P, 3], bf)
            nc.sync.dma_start(out=a_t[:], in_=att[:])
            ones = cst.tile([1, P], bf)
            nc.vector.memset(ones[:], 1.0)

            for (c0, w) in GROUPS:
                s_t = [sb.tile([P, w], bf, tag=f"s{k}", name=f"s_t{k}")
                       for k in range(3)]
                for k in range(3):
                    nc.sync.dma_start_transpose(out=s_t[k][:],
                                                in_=sD[k][c0:c0 + w, :])
                yps = [ps.tile([P, GB], f32, tag=f"y{k}", name=f"yps{k}")
                       for k in range(3)]
                y_sb = [sb.tile([P, w], bf, tag=f"ysb{k}", name=f"y_sb{k}")
                        for k in range(3)]
                for k in range(3):
                    nc.tensor.matmul(out=yps[k][:, :w],
                                     lhsT=wt_t[:, k * D:(k + 1) * D],
                                     rhs=s_t[k][:], start=True, stop=True)
                    nc.scalar.activation(out=y_sb[k][:], in_=yps[k][:, :w],
                                         func=Relu, bias=b_t[:, k:k + 1],
                                         scale=1.0)
                scp = ps.tile([P, GB], f32, tag="sc")
                e_sb = sb.tile([1, 3 * w], f32, tag="esb")
                for k in range(3):
                    nc.tensor.matmul(out=scp[0:1, :w],
                                     lhsT=a_t[:, k:k + 1],
                                     rhs=y_sb[k][:], start=True, stop=True)
                    nc.scalar.activation(out=e_sb[0:1, k * w:(k + 1) * w],
                                         in_=scp[0:1, :w], func=Exp)
                den = sb.tile([1, w], f32, tag="den")
                nc.vector.tensor_tensor(out=den[:], in0=e_sb[0:1, 0:w],
                                        in1=e_sb[0:1, w:2 * w], op=Add)
                nc.vector.tensor_tensor(out=den[:], in0=den[:],
                                        in1=e_sb[0:1, 2 * w:3 * w], op=Add)
                rec = sb.tile([1, w], f32, tag="rec")
                nc.vector.reciprocal(out=rec[:], in_=den[:])
                w_sb = sb.tile([1, 3 * w], bf, tag="wsb")
                for k in range(3):
                    nc.vector.tensor_tensor(
                        out=w_sb[0:1, k * w:(k + 1) * w],
                        in0=e_sb[0:1, k * w:(k + 1) * w],
                        in1=rec[:], op=Mult)
                acc = sb.tile([P, w], bf, tag="acc")
                tmp = sb.tile([P, w], bf, tag="tmp")
                for k in range(3):
                    wbp = ps.tile([P, GB], f32, tag=f"wb{k}", name=f"wbp{k}")
                    nc.tensor.matmul(out=wbp[:, :w], lhsT=ones[:],
                                     rhs=w_sb[0:1, k * w:(k + 1) * w],
                                     start=True, stop=True)
                    dst = acc if k == 0 else tmp
                    nc.vector.tensor_tensor(out=dst[:], in0=y_sb[k][:],
                                            in1=wbp[:, :w], op=Mult)
                    if k > 0:
                        nc.vector.tensor_tensor(out=acc[:], in0=acc[:],
                                                in1=tmp[:], op=Add)
                nc.sync.dma_start(out=outT[:, c0:c0 + w], in_=acc[:])
    nc.compile()
    return nc


# --------------------------------------------------------------------------
# cached PJRT dispatch (mirrors bass2jax.run_bass_via_pjrt, jit built once)
# --------------------------------------------------------------------------

def _enable_jax_cache():
    # persistent XLA compilation cache: a fresh process skips the
    # shard_map/zeros jit compiles (~15 s) on its first call
    try:
        import jax
        cache_dir = "/var/tmp/magnn_jax_cache"
        os.makedirs(cache_dir, exist_ok=True)
        jax.config.update("jax_compilation_cache_dir", cache_dir)
        jax.config.update("jax_persistent_cache_min_entry_size_bytes", -1)
        jax.config.update("jax_persistent_cache_min_compile_time_secs", 0)
    except Exception:
        pass


def _build_dispatch(nc):
    import jax
    from jax.experimental.shard_map import shard_map
    from jax.sharding import Mesh, PartitionSpec, NamedSharding
    import concourse.mybir as mybir
    from concourse import bass2jax

    _enable_jax_cache()
    bass2jax.install_neuronx_cc_hook()

    partition_name = (nc.partition_id_tensor.name
                      if nc.partition_id_tensor else None)
    in_names, out_names, out_avals, zero_outs = [], [], [], []
    for alloc in nc.m.functions[0].allocations:
        if not isinstance(alloc, mybir.MemoryLocationSet):
            continue
        name = alloc.memorylocations[0].name
        if alloc.kind == "ExternalInput":
            if name != partition_name:
                in_names.append(name)
        elif alloc.kind == "ExternalOutput":
            shape = tuple(alloc.tensor_shape)
            dtype = mybir.dt.np(alloc.dtype)
            out_names.append(name)
            out_avals.append(jax.core.ShapedArray(shape, dtype))
            zero_outs.append(
                np.zeros((NCORES * shape[0],) + shape[1:], dtype))
    n_params = len(in_names)
    all_names = list(in_names) + list(out_names)
    if partition_name is not None:
        all_names.append(partition_name)
    donate = tuple(range(n_params, n_params + len(out_names)))

    def _body(*args):
        operands = list(args)
        if partition_name is not None:
            operands.append(bass2jax.partition_id_tensor())
        outs = bass2jax._bass_exec_p.bind(
            *operands,
            out_avals=tuple(out_avals),
            in_names=tuple(all_names),
            out_names=tuple(out_names),
            lowering_input_output_aliases=(),
            sim_require_finite=True,
            sim_require_nnan=True,
            nc=nc,
        )
        return tuple(outs)

    import jax.numpy as jnp
    devices = jax.devices()[:NCORES]
    spec = PartitionSpec("core")
    n_args = n_params + len(out_names)
    zspecs = [(tuple(a.shape), a.dtype) for a in out_avals]

    # NSPLIT independent sub-mesh dispatches over the SAME bass program.
    # Split i launches as soon as its input rows have landed, and its
    # output fetch (downstream) overlaps the later splits' input puts
    # (upstream) — the axon tunnel is full-duplex.
    ndev = NCORES // NSPLIT
    splits = []
    for lo in range(0, NCORES, ndev):
        mesh_h = Mesh(np.asarray(devices[lo:lo + ndev]), ("core",))
        sharding_h = NamedSharding(mesh_h, spec)
        fn_h = jax.jit(
            shard_map(_body, mesh=mesh_h, in_specs=(spec,) * n_args,
                      out_specs=(spec,) * len(out_names), check_rep=False),
            donate_argnums=donate, keep_unused=True)

        def _mk_zeros(_zspecs=zspecs, _ndev=ndev):
            return tuple(jnp.zeros((_ndev * s[0],) + s[1:], d)
                         for s, d in _zspecs)

        zeros_fn_h = jax.jit(_mk_zeros,
                             out_shardings=(sharding_h,) * len(zero_outs))
        splits.append({"fn": fn_h, "sharding": sharding_h,
                       "zeros_fn": zeros_fn_h})
    return {
        "splits": splits,
        "in_names": in_names,
        "devices": list(devices),
    }


# --------------------------------------------------------------------------
# host: CSR graph state + preallocated buffers
# --------------------------------------------------------------------------

def _fingerprint(*arrs):
    return tuple(
        (a.shape[0], float(np.asarray(a[::257]).astype(np.float64).sum()))
        for a in arrs
    )


def _build_host(ei1_src, ei1_dst, ei2_src, ei2_dst, ei12_src, ei12_dst,
                ew1, ew2):
    import scipy.sparse as sp

    ei1_src = np.asarray(ei1_src)
    ei1_dst = np.asarray(ei1_dst)
    ei2_src = np.asarray(ei2_src)
    ei2_dst = np.asarray(ei2_dst)
    ei12_src = np.asarray(ei12_src)
    ei12_dst = np.asarray(ei12_dst)

    def recip_counts(idx, size):
        c = np.bincount(idx, minlength=size).astype(np.float32)
        np.maximum(c, 1.0, out=c)
        np.reciprocal(c, out=c)
        return c

    # All per-row scalings (segment-mean 1/cnt and the (msg+x)*0.5
    # halving) are folded into the static CSR data, so the per-call
    # pipeline is pure SpMM + one add per stage:
    #   m1  = A1 @ x_node        (= msg1, mean already applied)
    #   m1 += x1                 (un-halved net1; 0.5 lives in U1/B12)
    #   s1  = U1 @ m1            (= s1s_pre)
    #   ... analogous for metapaths 2 and 1-2
    rD1 = recip_counts(ei1_dst, N1)
    rD2 = recip_counts(ei2_dst, N2)
    rD12 = recip_counts(ei12_dst, N2)
    rC1 = recip_counts(ei1_src, N0)
    rC2 = recip_counts(ei2_src, N0)
    ew1 = np.asarray(ew1, np.float32)
    ew2 = np.asarray(ew2, np.float32)

    # Column-blocked CSR: each block's gathers hit a cache-resident
    # slice of the source matrix (x_node is 51 MB; 16 blocks -> 3.2 MB
    # slices; measured 2.2x faster than one unblocked SpMM). Returns
    # [(csr, xlo, xhi), ...]; consumers accumulate over blocks.
    def col_blocks(row, col, dat, nrows, ncols, nb):
        W = ncols // nb
        order = np.argsort(col, kind="stable")
        r, c, v = row[order], col[order], dat[order]
        bounds = np.searchsorted(c, np.arange(0, ncols + 1, W))
        return [
            (sp.csr_matrix((v[lo:hi], (r[lo:hi], c[lo:hi] - b * W)),
                           shape=(nrows, W)), b * W, (b + 1) * W)
            for b, (lo, hi) in enumerate(zip(bounds[:-1], bounds[1:]))
        ]

    st = {
        "A1": col_blocks(ei1_dst, ei1_src, ew1 * rD1[ei1_dst],
                         N1, N0, 16),
        "A2": col_blocks(ei2_dst, ei2_src, ew2 * rD2[ei2_dst],
                         N2, N0, 16),
        "B12": col_blocks(ei12_dst, ei12_src, 0.5 * rD12[ei12_dst],
                          N2, N1, 2),
        "U1": col_blocks(ei1_src, ei1_dst, 0.5 * rC1[ei1_src],
                         N0, N1, 2),
        "U2": col_blocks(ei2_src, ei2_dst, 0.5 * rC2[ei2_src],
                         N0, N2, 2),
        "V2": col_blocks(ei2_src, ei2_dst, 0.5 * ew2 * rC2[ei2_src],
                         N0, N2, 2),
    }

    # per-core row blocks of the (column-blocked) N0-output CSRs, for
    # streamed compute+put: blocks[c] = [(r0, r1, ip, idx, dat, xlo,
    # xhi), ...] — one entry per column block, accumulated in order
    def row_blocks(col_blocked):
        blocks = []
        for c in range(NCORES):
            pieces = []
            for (A, xlo, xhi) in col_blocked:
                r0, r1 = c * ROWS, min((c + 1) * ROWS, A.shape[0])
                ip = (A.indptr[r0:r1 + 1] -
                      A.indptr[r0]).astype(A.indptr.dtype)
                lo, hi = A.indptr[r0], A.indptr[r1]
                pieces.append((r0, r1, ip, A.indices[lo:hi],
                               A.data[lo:hi], xlo, xhi))
            blocks.append(pieces)
        return blocks

    st["U1b"] = row_blocks(st["U1"])
    st["U2b"] = row_blocks(st["U2"])
    st["V2b"] = row_blocks(st["V2"])
    # preallocated, page-warmed buffers
    for nm, shape, dt in (
            ("m1", (N1, D), np.float32), ("m2", (N2, D), np.float32),
            ("m2b", (N2, D), np.float32),
            ("sp1", (N0P, D), np.float32), ("sp2", (N0P, D), np.float32),
            ("sp12", (N0P, D), np.float32),
            ("sb1", (N0P, D), BF16), ("sb2", (N0P, D), BF16),
            ("sb3", (N0P, D), BF16),
            ("outA", (N0P, D), np.float32), ("outB", (N0P, D), np.float32)):
        b = np.zeros(shape, dt)
        b.reshape(-1)[::1024] = 0          # fault the pages in now
        st[nm] = b
    return st


def _spmm(col_blocked, X, out):
    """out = A @ X for a column-blocked CSR, into a preallocated buffer
    (csr_matvecs accumulates, so blocks just chain)."""
    from scipy.sparse import _sparsetools
    out.fill(0)
    for (A, xlo, xhi) in col_blocked:
        _sparsetools.csr_matvecs(A.shape[0], A.shape[1], X.shape[1],
                                 A.indptr, A.indices, A.data,
                                 X[xlo:xhi], out.ravel())


# --------------------------------------------------------------------------
# entry point
# --------------------------------------------------------------------------

def kernel(x_node, x1, x2, ei1_src, ei1_dst, ei2_src, ei2_dst,
           ei12_src, ei12_dst, ew1, ew2,
           W1, b1, W2, b2, W12, b12, att_vec):
    global LAST_EXEC_NS
    import time as _time
    import jax
    from concourse.bass_utils import axon_active

    _dbg = bool(int(os.environ.get("MAGNN_DEBUG", "0")))
    _t0 = _time.time()

    def _lap(msg):
        if _dbg:
            print(f"    [kernel] {msg}: {_time.time() - _t0:.2f}s",
                  flush=True)

    x_node = np.ascontiguousarray(x_node, np.float32)
    x1 = np.ascontiguousarray(x1, np.float32)
    x2 = np.ascontiguousarray(x2, np.float32)
    ew1 = np.asarray(ew1, np.float32)
    ew2 = np.asarray(ew2, np.float32)

    if "prog" not in _C:
        _C["prog"] = _build_program()
    nc = _C["prog"]
    use_fast = axon_active()
    if use_fast and "disp" not in _C:
        _C["disp"] = _build_dispatch(nc)
    _lap("program+dispatch ready")

    fp = _fingerprint(ei1_src, ei1_dst, ei2_src, ei2_dst,
                      ei12_src, ei12_dst, ew1, ew2)
    if _C.get("host_fp") != fp:
        _C["host"] = _build_host(ei1_src, ei1_dst, ei2_src, ei2_dst,
                                 ei12_src, ei12_dst, ew1, ew2)
        _C["host_fp"] = fp
        _C["out_flip"] = False
    h = _C["host"]
    _lap("host state ready")

    if use_fast:
        disp = _C["disp"]
        splits = disp["splits"]

    # small replicated params, packed/put lazily (after the first s1 put
    # is on the wire — this call-start CPU would otherwise delay it)
    def _prep_params():
        wt = np.concatenate(
            [np.ascontiguousarray(np.asarray(W).T) for W in (W1, W2, W12)],
            axis=1).astype(BF16)
        bias = np.stack([np.asarray(b1), np.asarray(b2), np.asarray(b12)],
                        axis=1).astype(np.float32)
        att = np.ascontiguousarray(np.asarray(att_vec).T).astype(BF16)
        if not use_fast:
            return wt, bias, att, None, None
        wfp = (wt.tobytes(), bias.tobytes(), att.tobytes())
        if _C.get("w_fp") != wfp:
            for grp in _C.pop("w_dev", ()):
                for a in grp:
                    try:
                        a.delete()
                    except Exception:
                        pass
            nh = NCORES // NSPLIT
            _C["w_dev"] = tuple(
                tuple(jax.device_put(np.tile(a, (nh, 1)), s["sharding"])
                      for a in (wt, bias, att))
                for s in splits)
            _C["w_fp"] = wfp
        zeros_devs = [s["zeros_fn"]()[0] for s in splits]
        return wt, bias, att, _C["w_dev"], zeros_devs

    # ---- host: segment-mean pipeline as CSR SpMM, overlapped with puts ----
    from scipy.sparse import _sparsetools
    m1, m2, m2b = h["m1"], h["m2"], h["m2b"]
    CPS = NCORES // NSPLIT               # cores per split
    SROWS = CPS * ROWS                   # global rows per split

    def stream_s(blocks, X, sp, sb, on_part=None, fine=False):
        """Per-core row block: SpMM -> bf16; put each sub-mesh's rows as
        soon as they are done (the wire drains split 0 while the later
        splits are still being computed). `on_part(i, dev_array)` fires
        right after split i's put is issued. With fine=True each core's
        rows go on the wire individually (worth the extra put overhead
        only for the first stream, when the wire is otherwise idle).
        """
        parts = []
        pend = []
        for c, pieces in enumerate(blocks):
            r0, r1 = pieces[0][0], pieces[0][1]
            blk = sp[r0:r1]
            blk.fill(0)
            for (r0, r1, ip, idx, dat, xlo, xhi) in pieces:
                _sparsetools.csr_matvecs(r1 - r0, xhi - xlo, D, ip, idx,
                                         dat, X[xlo:xhi], blk.ravel())
            np.copyto(sb[r0:r1], blk, casting="unsafe")
            if not use_fast:
                continue
            if fine:
                pend.append(jax.device_put(sb[c * ROWS:(c + 1) * ROWS],
                                           disp["devices"][c]))
            if (c + 1) % CPS == 0:
                i = c // CPS
                if fine:
                    dv = jax.make_array_from_single_device_arrays(
                        (SROWS, D), splits[i]["sharding"], pend)
                    pend = []
                else:
                    dv = jax.device_put(sb[i * SROWS:(i + 1) * SROWS],
                                        splits[i]["sharding"])
                parts.append(dv)
                if on_part is not None:
                    on_part(i, dv)
        return parts if use_fast else None

    _spmm(h["A1"], x_node, m1)           # msg1 (mean folded into A1)
    m1 += x1                             # un-halved net1 (0.5 in U1/B12)
    # fine=True (per-core puts) measured net-negative here: the ~0.1 s
    # earlier wire start is outweighed by the extra per-put CPU cost on
    # this single-core host
    d1 = stream_s(h["U1b"], m1, h["sp1"], h["sb1"])             # s1s_pre
    _lap("s1 ready+put")
    wt, bias, att, w_split, zeros_devs = _prep_params()
    _lap("params ready")

    _spmm(h["A2"], x_node, m2)           # msg2
    m2 += x2                             # un-halved net2 (0.5 in U2)
    d2 = stream_s(h["U2b"], m2, h["sp2"], h["sb2"])             # s2s_pre
    _lap("s2 ready+put")

    # s12s: dispatch split i the moment its s3 rows are on the wire; its
    # output fetch (downstream) overlaps later splits' puts (upstream)
    outs_split = [None] * NSPLIT

    def _launch(i, d3_h):
        wth, biash, atth = w_split[i]
        arg_map = {"s0": d1[i], "s1": d2[i], "s2": d3_h,
                   "wt": wth, "bias": biash, "att": atth}
        args = [arg_map[n] for n in disp["in_names"]] + [zeros_devs[i]]
        outs_split[i] = (splits[i]["fn"](*args), d3_h)
        try:
            outs_split[i][0][0].copy_to_host_async()
        except Exception:
            pass
        _lap(f"split {i} dispatched")

    _spmm(h["B12"], m1, m2b)             # msg2b from net1
    m2b += x2                            # un-halved net2b (0.5 in V2)
    stream_s(h["V2b"], m2b, h["sp12"], h["sb3"],
             on_part=_launch if use_fast else None)              # s12s_pre
    _lap("s3 ready+put")

    # ---- device: linear + relu + attention softmax combine ----
    out = h["outB"] if _C["out_flip"] else h["outA"]
    _C["out_flip"] = not _C["out_flip"]

    if use_fast:
        # fetch split i, then transpose it into `out` while split i+1's
        # fetch (started via copy_to_host_async) is still streaming
        for i in range(NSPLIT):
            outs, _ = outs_split[i]
            pc = np.asarray(outs[0]).reshape(CPS, P, ROWS)
            for j in range(CPS):
                c = i * CPS + j
                np.copyto(out[c * ROWS:(c + 1) * ROWS, :],
                          pc[j].T, casting="unsafe")
        _lap("output fetched+transposed")
        # free device buffers last, so dealloc chatter can't stall the
        # next call's transfers
        for i in range(NSPLIT):
            outs, d3_h = outs_split[i]
            for a in (d1[i], d2[i], d3_h, outs[0]):
                try:
                    a.delete()
                except Exception:
                    pass
    else:
        from concourse.bass_utils import run_bass_kernel_spmd
        in_maps = []
        for c in range(NCORES):
            rows = slice(c * ROWS, (c + 1) * ROWS)
            in_maps.append({
                "s0": np.ascontiguousarray(h["sb1"][rows]),
                "s1": np.ascontiguousarray(h["sb2"][rows]),
                "s2": np.ascontiguousarray(h["sb3"][rows]),
                "wt": wt, "bias": bias, "att": att})
        res = run_bass_kernel_spmd(nc, in_maps, list(range(NCORES)),
                                   trace=False)
        LAST_EXEC_NS = res.exec_time_ns
        for c in range(NCORES):
            np.copyto(out[c * ROWS:(c + 1) * ROWS, :],
                      res.results[c]["outT"].T, casting="unsafe")
    _lap("done")
    return out[:N0]



# revision 2
# speedup vs baseline: 1.7987x; 1.7987x over previous
"""MAGNN aggregation kernel — all graph compute on one TRN2 NeuronCore.

The host's only per-call work is int8-quantizing the feature matrices
(x_node|x1|x2 -> one [200192, 128] int8 buffer, ~25.6 MB on the wire) and
dequantizing the int8 + per-row-scale output (~13 MB back).  Everything
else — the six gather / segment-mean stages of the three metapaths and the
linear+relu+attention epilogue — runs on device 0:

  stage        edges    op
  A1/A2        1.6M ea  m_k   = scatter_mean(x_node[src]*ew, dst)      (*)
  B12          0.8M     m_2b  = scatter_mean(net1[src], dst)
  U1/U2/V2     1.6M ea  s_k   = scatter_mean(net_k[dst], src) -> [N0]
  epilogue              y_k = relu(W_k s_k + b_k); softmax-att combine

  (*) per-edge segment sums are computed as one-hot matmuls: for each tile
  of 128 destination rows, gather the (padded) edge source rows with
  indirect DMAs of 128 rows each, build the 128x128 selection matrix
  Sel[e, d] = w_e * (dloc_e == d) on the vector engine, and accumulate
  psum += Sel.T @ G on the tensor engine.

The edge tables (src index / folded weight / dst-local offset per padded
edge slot) and the packed linear/attention params are baked into the NEFF
as inline consts — they ride along at model-load time, so a warm call
ships ONLY the 25.6 MB of features.  Graph or param changes are detected
by fingerprint and trigger a program rebuild (slow, but unchanged inputs
never pay it).  scatter_mean counts, the *0.5 factors and the int8
dequant 1/S are all folded into the per-edge weights at table-build time.
"""
import os
import numpy as np
import ml_dtypes

BF16 = ml_dtypes.bfloat16

S = 31.75          # int8 quant scale for x  (clips at ~4 sigma)
N0, N1, N2 = 100000, 50000, 50000
N0P = 100096       # 782 * 128
N1P = 50048        # 391 * 128
N2P = 50048
XQ_ROWS = N0P + N1P + N2P
X1_BASE = N0P
X2_BASE = N0P + N1P
D = 128
GB = 256           # epilogue group width

_C = {}
LAST_EXEC_NS = None


# ---------------------------------------------------------------- host tables

def _stage_tables(dst, src, w, T):
    """Pack one stage's edges into [T*128, K] tables: element (t*128+p, c)
    = edge (tile t, chunk c, lane p); values src-row / folded-weight /
    dst-local-offset.  Padding slots: idx 0, w 0."""
    dst = np.asarray(dst); src = np.asarray(src); w = np.asarray(w, np.float32)
    tile_id = dst >> 7
    dloc = (dst & 127).astype(np.int8)
    order = np.argsort(tile_id, kind="stable")
    cnt = np.bincount(tile_id, minlength=T)
    K = int(np.ceil(cnt.max() / 128.0))
    W = K * 128
    starts = np.zeros(T, np.int64)
    np.cumsum(cnt[:-1], out=starts[1:])
    col = np.arange(len(dst), dtype=np.int64) - np.repeat(starts, cnt)
    row = tile_id[order]
    idx_p = np.zeros((T, W), np.int32)
    w_p = np.zeros((T, W), BF16)
    dl_p = np.zeros((T, W), np.int8)
    idx_p[row, col] = src[order]
    w_p[row, col] = w[order].astype(BF16)
    dl_p[row, col] = dloc[order]

    def relay(a):  # [T, K*128] -> [T*128, K]
        return np.ascontiguousarray(
            a.reshape(T, K, 128).transpose(0, 2, 1).reshape(T * 128, K))
    return relay(idx_p), relay(w_p), relay(dl_p), K


def _build_tables(ei1_src, ei1_dst, ei2_src, ei2_dst, ei12_src, ei12_dst,
                  ew1, ew2):
    def recip_counts(idx, size):
        c = np.bincount(idx, minlength=size).astype(np.float32)
        return 1.0 / np.maximum(c, 1.0)

    rD1 = recip_counts(ei1_dst, N1)
    rD2 = recip_counts(ei2_dst, N2)
    rD12 = recip_counts(ei12_dst, N2)
    rC1 = recip_counts(ei1_src, N0)
    rC2 = recip_counts(ei2_src, N0)
    ew1 = np.asarray(ew1, np.float32)
    ew2 = np.asarray(ew2, np.float32)
    T1, T0 = N1P // 128, N0P // 128
    return {
        # A-stages gather int8 x rows; fold the 1/S dequant into the weights
        "A1": _stage_tables(ei1_dst, ei1_src, ew1 * rD1[ei1_dst] / S, T1),
        "A2": _stage_tables(ei2_dst, ei2_src, ew2 * rD2[ei2_dst] / S, T1),
        "B12": _stage_tables(ei12_dst, ei12_src, 0.5 * rD12[ei12_dst], T1),
        "U1": _stage_tables(ei1_src, ei1_dst, 0.5 * rC1[ei1_src], T0),
        "U2": _stage_tables(ei2_src, ei2_dst, 0.5 * rC2[ei2_src], T0),
        "V2": _stage_tables(ei2_src, ei2_dst, 0.5 * ew2 * rC2[ei2_src], T0),
    }


def _pack_params(W1, b1, W2, b2, W12, b12, att_vec):
    par = np.zeros((128, 390), BF16)
    for k, Wk in enumerate((W1, W2, W12)):
        par[:, k * D:(k + 1) * D] = np.asarray(Wk).T.astype(BF16)
    for k, b in enumerate((b1, b2, b12)):
        par[:, 384 + k] = np.asarray(b).astype(BF16)
    par[:, 387:390] = np.ascontiguousarray(np.asarray(att_vec).T).astype(BF16)
    return par


# ------------------------------------------------------------- device program

def _build_program(tbl, par_h):
    import concourse.bacc as bacc
    import concourse.mybir as mybir
    import concourse.tile as tile
    from concourse.bass import ds, IndirectOffsetOnAxis

    nc = bacc.Bacc("TRN2", target_bir_lowering=False, debug=False,
                   num_devices=1)
    bf = mybir.dt.bfloat16
    f32 = mybir.dt.float32
    i8 = mybir.dt.int8
    i32 = mybir.dt.int32
    Relu = mybir.ActivationFunctionType.Relu
    Exp = mybir.ActivationFunctionType.Exp
    Copy = mybir.ActivationFunctionType.Copy
    Mult = mybir.AluOpType.mult
    Add = mybir.AluOpType.add
    Eq = mybir.AluOpType.is_equal
    Max = mybir.AluOpType.max

    xq = nc.dram_tensor("xq", [XQ_ROWS, D], i8, kind="ExternalInput")
    outq = nc.dram_tensor("outq", [N0P, D], i8, kind="ExternalOutput")
    rowscale = nc.dram_tensor("rowscale", [N0P, 1], f32,
                              kind="ExternalOutput")
    net1 = nc.dram_tensor("net1", [N1P, D], bf, kind="Internal")
    net2 = nc.dram_tensor("net2", [N2P, D], bf, kind="Internal")
    net2b = nc.dram_tensor("net2b", [N2P, D], bf, kind="Internal")
    s1T = nc.dram_tensor("s1T", [D, N0P], bf, kind="Internal")
    s2T = nc.dram_tensor("s2T", [D, N0P], bf, kind="Internal")
    s12T = nc.dram_tensor("s12T", [D, N0P], bf, kind="Internal")

    const = {}
    for snm in ("A1", "A2", "B12", "U1", "U2", "V2"):
        idx_a, w_a, dl_a, K = tbl[snm]
        const[snm] = (nc.inline_tensor(idx_a, f"idx_{snm}"),
                      nc.inline_tensor(w_a, f"w_{snm}"),
                      nc.inline_tensor(dl_a, f"dl_{snm}"), K)
    iota_c = nc.inline_tensor(
        np.tile(np.arange(128, dtype=np.float32).astype(BF16), (128, 1)),
        "iota_bf")
    ident_c = nc.inline_tensor(np.eye(128, dtype=BF16), "ident_bf")
    par_c = nc.inline_tensor(np.ascontiguousarray(par_h), "par")

    def gather_stage(tc, sb, ps, cst, snm, src_dram, src_is_i8,
                     T, x_base, net_out, sT_out):
        idx_c, w_c, dl_c, K = const[snm]
        iota_t = cst["iota"]
        with tc.For_i(0, T * 128, 128, name=f"st_{snm}") as i:
            idx_t = sb.tile([128, K], i32, tag="idx")
            nc.sync.dma_start(out=idx_t[:], in_=idx_c[ds(i, 128), :])
            w_t = sb.tile([128, K], bf, tag="w")
            nc.sync.dma_start(out=w_t[:], in_=w_c[ds(i, 128), :])
            dl_t = sb.tile([128, K], i8, tag="dl")
            nc.sync.dma_start(out=dl_t[:], in_=dl_c[ds(i, 128), :])
            dl_b = sb.tile([128, K], bf, tag="dlb")
            nc.vector.tensor_copy(out=dl_b[:], in_=dl_t[:])

            if src_is_i8:
                g8 = sb.tile([128, K * D], i8, tag="g8")
                for c in range(K):
                    nc.gpsimd.indirect_dma_start(
                        out=g8[:, c * D:(c + 1) * D], out_offset=None,
                        in_=src_dram[:],
                        in_offset=IndirectOffsetOnAxis(
                            ap=idx_t[:, c:c + 1], axis=0))
                gb = sb.tile([128, K * D], bf, tag="gb")
                nc.scalar.activation(out=gb[:], in_=g8[:], func=Copy)
            else:
                gb = sb.tile([128, K * D], bf, tag="gb")
                for c in range(K):
                    nc.gpsimd.indirect_dma_start(
                        out=gb[:, c * D:(c + 1) * D], out_offset=None,
                        in_=src_dram[:],
                        in_offset=IndirectOffsetOnAxis(
                            ap=idx_t[:, c:c + 1], axis=0))

            acc = ps.tile([128, D], f32, tag="acc")
            for c in range(K):
                eq = sb.tile([128, D], bf, tag="eq")
                nc.vector.tensor_tensor(
                    out=eq[:], in0=dl_b[:, c:c + 1].to_broadcast([128, D]),
                    in1=iota_t[:], op=Eq)
                sel = sb.tile([128, D], bf, tag="sel")
                nc.vector.tensor_tensor(
                    out=sel[:], in0=eq[:],
                    in1=w_t[:, c:c + 1].to_broadcast([128, D]), op=Mult)
                nc.tensor.matmul(out=acc[:], lhsT=sel[:],
                                 rhs=gb[:, c * D:(c + 1) * D],
                                 start=(c == 0), stop=(c == K - 1))

            if net_out is not None:
                xt8 = sb.tile([128, D], i8, tag="xt8")
                nc.sync.dma_start(out=xt8[:], in_=xq[ds(i + x_base, 128), :])
                xtb = sb.tile([128, D], bf, tag="xtb")
                nc.scalar.activation(out=xtb[:], in_=xt8[:], func=Copy,
                                     scale=1.0 / S)
                net_sb = sb.tile([128, D], bf, tag="net")
                nc.vector.tensor_tensor(out=net_sb[:], in0=acc[:],
                                        in1=xtb[:], op=Add)
                nc.sync.dma_start(out=net_out[ds(i, 128), :], in_=net_sb[:])
            else:
                s_sb = sb.tile([128, D], bf, tag="ssb")
                nc.scalar.activation(out=s_sb[:], in_=acc[:], func=Copy)
                sT_ps = ps.tile([128, D], bf, tag="sT")
                nc.tensor.transpose(out=sT_ps[:], in_=s_sb[:],
                                    identity=cst["ident"][:])
                sT_sb = sb.tile([128, D], bf, tag="sTsb")
                nc.scalar.activation(out=sT_sb[:], in_=sT_ps[:], func=Copy)
                nc.sync.dma_start(out=sT_out[:, ds(i, 128)], in_=sT_sb[:])

    stages = [
        ("A1", xq, True, N1P // 128, X1_BASE, net1, None),
        ("A2", xq, True, N2P // 128, X2_BASE, net2, None),
        ("B12", net1, False, N2P // 128, X2_BASE, net2b, None),
        ("U1", net1, False, N0P // 128, None, None, s1T),
        ("U2", net2, False, N0P // 128, None, None, s2T),
        ("V2", net2b, False, N0P // 128, None, None, s12T),
    ]
    for (snm, src, is8, T, xb, no, so) in stages:
        with tile.TileContext(nc) as tc:
            with tc.tile_pool(name="cst", bufs=1) as cstp, \
                 tc.tile_pool(name="sb", bufs=2) as sb, \
                 tc.tile_pool(name="ps", bufs=2, space="PSUM") as ps:
                iota_t = cstp.tile([128, 128], bf, tag="iota")
                nc.sync.dma_start(out=iota_t[:], in_=iota_c[:])
                ident_t = cstp.tile([128, 128], bf, tag="ident")
                nc.sync.dma_start(out=ident_t[:], in_=ident_c[:])
                cst = {"iota": iota_t, "ident": ident_t}
                gather_stage(tc, sb, ps, cst, snm, src, is8, T, xb, no, so)

    # ---------------- epilogue ----------------
    with tile.TileContext(nc) as tc:
        with tc.tile_pool(name="sb", bufs=2) as sb, \
             tc.tile_pool(name="cst", bufs=1) as cst, \
             tc.tile_pool(name="ps", bufs=2, space="PSUM") as ps:
            par_t = cst.tile([128, 390], bf, tag="par")
            nc.sync.dma_start(out=par_t[:], in_=par_c[:])
            ident_t = cst.tile([128, 128], bf, tag="ident")
            nc.sync.dma_start(out=ident_t[:], in_=ident_c[:])
            ones = cst.tile([1, 128], bf, tag="ones")
            nc.vector.memset(ones[:], 1.0)
            bias_f = cst.tile([128, 3], f32, tag="biasf")
            nc.vector.tensor_copy(out=bias_f[:], in_=par_t[:, 384:387])

            sTs = (s1T, s2T, s12T)
            from concourse.bass import ds as _ds
            with tc.For_i(0, N0P, GB, name="epi") as j:
                y = []
                for k in range(3):
                    sk = sb.tile([128, GB], bf, tag=f"s{k}")
                    nc.sync.dma_start(out=sk[:], in_=sTs[k][:, _ds(j, GB)])
                    yp = ps.tile([128, GB], f32, tag="yp")
                    nc.tensor.matmul(out=yp[:],
                                     lhsT=par_t[:, k * D:(k + 1) * D],
                                     rhs=sk[:], start=True, stop=True)
                    yk = sb.tile([128, GB], bf, tag=f"y{k}")
                    nc.scalar.activation(out=yk[:], in_=yp[:], func=Relu,
                                         bias=bias_f[:, k:k + 1], scale=1.0)
                    y.append(yk)
                e_sb = sb.tile([1, 3 * GB], f32, tag="esb")
                for k in range(3):
                    scp = ps.tile([1, GB], f32, tag="sc")
                    nc.tensor.matmul(out=scp[:],
                                     lhsT=par_t[:, 387 + k:388 + k],
                                     rhs=y[k][:], start=True, stop=True)
                    nc.scalar.activation(out=e_sb[0:1, k * GB:(k + 1) * GB],
                                         in_=scp[:], func=Exp)
                den = sb.tile([1, GB], f32, tag="den")
                nc.vector.tensor_tensor(out=den[:], in0=e_sb[0:1, 0:GB],
                                        in1=e_sb[0:1, GB:2 * GB], op=Add)
                nc.vector.tensor_tensor(out=den[:], in0=den[:],
                                        in1=e_sb[0:1, 2 * GB:3 * GB], op=Add)
                rec = sb.tile([1, GB], f32, tag="rec")
                nc.vector.reciprocal(out=rec[:], in_=den[:])
                w_sb = sb.tile([1, 3 * GB], bf, tag="wsb")
                for k in range(3):
                    nc.vector.tensor_tensor(
                        out=w_sb[0:1, k * GB:(k + 1) * GB],
                        in0=e_sb[0:1, k * GB:(k + 1) * GB], in1=rec[:],
                        op=Mult)
                acc = sb.tile([128, GB], bf, tag="acc")
                tmp = sb.tile([128, GB], bf, tag="tmp")
                for k in range(3):
                    wbp = ps.tile([128, GB], f32, tag="wb")
                    nc.tensor.matmul(out=wbp[:], lhsT=ones[:],
                                     rhs=w_sb[0:1, k * GB:(k + 1) * GB],
                                     start=True, stop=True)
                    dst = acc if k == 0 else tmp
                    nc.vector.tensor_tensor(out=dst[:], in0=y[k][:],
                                            in1=wbp[:], op=Mult)
                    if k > 0:
                        nc.vector.tensor_tensor(out=acc[:], in0=acc[:],
                                                in1=tmp[:], op=Add)
                for sub in range(GB // 128):
                    aT_ps = ps.tile([128, 128], bf, tag="aT")
                    nc.tensor.transpose(out=aT_ps[:],
                                        in_=acc[:, sub * 128:(sub + 1) * 128],
                                        identity=ident_t[:])
                    rmax = sb.tile([128, 1], f32, tag="rmax")
                    nc.vector.tensor_reduce(
                        out=rmax[:], in_=aT_ps[:],
                        axis=mybir.AxisListType.XYZW, op=Max,
                        apply_absolute_value=True)
                    nc.vector.tensor_scalar_max(out=rmax[:], in0=rmax[:],
                                                scalar1=1e-20)
                    rcp = sb.tile([128, 1], f32, tag="rcp")
                    nc.vector.reciprocal(out=rcp[:], in_=rmax[:])
                    scl = sb.tile([128, 1], f32, tag="scl")
                    nc.vector.tensor_scalar_mul(out=scl[:], in0=rcp[:],
                                                scalar1=127.0)
                    qf = sb.tile([128, 128], f32, tag="qf")
                    nc.vector.tensor_tensor(
                        out=qf[:], in0=aT_ps[:],
                        in1=scl[:].to_broadcast([128, 128]), op=Mult)
                    qi = sb.tile([128, 128], i8, tag="qi")
                    nc.vector.tensor_copy(out=qi[:], in_=qf[:])
                    nc.sync.dma_start(
                        out=outq[_ds(j + sub * 128, 128), :], in_=qi[:])
                    sout = sb.tile([128, 1], f32, tag="sout")
                    nc.vector.tensor_scalar_mul(out=sout[:], in0=rmax[:],
                                                scalar1=1.0 / 127.0)
                    nc.sync.dma_start(
                        out=rowscale[_ds(j + sub * 128, 128), :],
                        in_=sout[:])
    nc.compile()
    return nc


# ---------------------------------------------------------- cached dispatch

def _enable_jax_cache():
    try:
        import jax
        cache_dir = "/var/tmp/magnn_jax_cache"
        os.makedirs(cache_dir, exist_ok=True)
        jax.config.update("jax_compilation_cache_dir", cache_dir)
        jax.config.update("jax_persistent_cache_min_entry_size_bytes", -1)
        jax.config.update("jax_persistent_cache_min_compile_time_secs", 0)
    except Exception:
        pass


def _build_dispatch(nc):
    import jax
    import jax.numpy as jnp
    import concourse.mybir as mybir
    from concourse import bass2jax

    _enable_jax_cache()
    bass2jax.install_neuronx_cc_hook()

    partition_name = (nc.partition_id_tensor.name
                      if nc.partition_id_tensor else None)
    in_names, out_names, out_avals = [], [], []
    for alloc in nc.m.functions[0].allocations:
        if not isinstance(alloc, mybir.MemoryLocationSet):
            continue
        name = alloc.memorylocations[0].name
        if alloc.kind == "ExternalInput":
            if name != partition_name:
                in_names.append(name)
        elif alloc.kind == "ExternalOutput":
            shape = tuple(alloc.tensor_shape)
            dtype = mybir.dt.np(alloc.dtype)
            out_names.append(name)
            out_avals.append(jax.core.ShapedArray(shape, dtype))
    n_params = len(in_names)
    all_names = list(in_names) + list(out_names)
    if partition_name is not None:
        all_names.append(partition_name)
    donate = tuple(range(n_params, n_params + len(out_names)))

    def _body(*args):
        operands = list(args)
        if partition_name is not None:
            operands.append(bass2jax.partition_id_tensor())
        outs = bass2jax._bass_exec_p.bind(
            *operands,
            out_avals=tuple(out_avals),
            in_names=tuple(all_names),
            out_names=tuple(out_names),
            lowering_input_output_aliases=(),
            sim_require_finite=True,
            sim_require_nnan=True,
            nc=nc,
        )
        return tuple(outs)

    dev = jax.devices()[0]
    fn = jax.jit(_body, donate_argnums=donate, keep_unused=True)

    zspecs = [(tuple(a.shape), a.dtype) for a in out_avals]

    def _mk_zeros():
        return tuple(jnp.zeros(s, d) for s, d in zspecs)

    zeros_fn = jax.jit(_mk_zeros)
    return {"fn": fn, "zeros_fn": zeros_fn, "in_names": in_names,
            "out_names": out_names, "device": dev}


# ------------------------------------------------------------------ host side

def _quantize_into(x_node, x1, x2, xq, tmp):
    for (x, base, n) in ((x_node, 0, N0), (x1, X1_BASE, N1),
                         (x2, X2_BASE, N2)):
        t = tmp[:n]
        np.multiply(x, S, out=t)
        np.rint(t, out=t)
        np.clip(t, -127, 127, out=t)
        np.copyto(xq[base:base + n], t, casting="unsafe")
    return xq


def _fingerprint(*arrs):
    parts = []
    for a in arrs:
        a = np.asarray(a)
        if a.size <= 66000:
            parts.append(a.tobytes())
        else:
            parts.append((a.shape[0],
                          float(np.asarray(a[::257], np.float64).sum()),
                          float(np.asarray(a[7::997], np.float64).sum())))
    return tuple(parts)


def kernel(x_node, x1, x2, ei1_src, ei1_dst, ei2_src, ei2_dst,
           ei12_src, ei12_dst, ew1, ew2,
           W1, b1, W2, b2, W12, b12, att_vec):
    global LAST_EXEC_NS
    import time as _time

    _dbg = bool(int(os.environ.get("MAGNN_DEBUG", "0")))
    _t0 = _time.time()

    def _lap(msg):
        if _dbg:
            print(f"    [kernel] {msg}: {_time.time() - _t0:.3f}s",
                  flush=True)

    x_node = np.ascontiguousarray(x_node, np.float32)
    x1 = np.ascontiguousarray(x1, np.float32)
    x2 = np.ascontiguousarray(x2, np.float32)

    fp = _fingerprint(ei1_src, ei1_dst, ei2_src, ei2_dst, ei12_src,
                      ei12_dst, ew1, ew2,
                      W1, b1, W2, b2, W12, b12, att_vec)
    if _C.get("fp") != fp:
        tbl = _build_tables(ei1_src, ei1_dst, ei2_src, ei2_dst,
                            ei12_src, ei12_dst, ew1, ew2)
        par = _pack_params(W1, b1, W2, b2, W12, b12, att_vec)
        _lap("tables built")
        _C.pop("disp", None)
        _C["prog"] = _build_program(tbl, par)
        _lap("program built")
        _C["fp"] = fp
        _C["xq"] = np.zeros((XQ_ROWS, D), np.int8)
        _C["tmp"] = np.zeros((N0, D), np.float32)
        _C["outA"] = np.zeros((N0P, D), np.float32)
        _C["outB"] = np.zeros((N0P, D), np.float32)
        _C["out_flip"] = False

    import jax
    from concourse.bass_utils import axon_active
    use_fast = axon_active()
    if use_fast and "disp" not in _C:
        _C["disp"] = _build_dispatch(_C["prog"])
        _lap("dispatch built")

    xq = _quantize_into(x_node, x1, x2, _C["xq"], _C["tmp"])
    _lap("quantized")

    out = _C["outB"] if _C["out_flip"] else _C["outA"]
    _C["out_flip"] = not _C["out_flip"]

    if use_fast:
        disp = _C["disp"]
        xq_dev = jax.device_put(xq, disp["device"])
        zeros = disp["zeros_fn"]()
        _lap("put issued")
        arg_map = {"xq": xq_dev}
        args = [arg_map[n] for n in disp["in_names"]] + list(zeros)
        outs = disp["fn"](*args)
        out_map = dict(zip(disp["out_names"], outs))
        oq, rs = out_map["outq"], out_map["rowscale"]
        try:
            oq.copy_to_host_async()
            rs.copy_to_host_async()
        except Exception:
            pass
        _lap("dispatched")
        oq_h = np.asarray(oq)
        rs_h = np.asarray(rs)
        _lap("fetched")
        np.copyto(out, oq_h, casting="unsafe")
        out *= rs_h
        _lap("dequantized")
        for a in (xq_dev, oq, rs):
            try:
                a.delete()
            except Exception:
                pass
    else:
        from concourse.bass_utils import run_bass_kernel_spmd
        res = run_bass_kernel_spmd(_C["prog"], [{"xq": xq}], [0],
                                   trace=False)
        LAST_EXEC_NS = res.exec_time_ns
        r = res.results[0]
        np.copyto(out, r["outq"], casting="unsafe")
        out *= r["rowscale"].astype(np.float32)
    _lap("done")
    return out[:N0]
